# revision 4
# baseline (speedup 1.0000x reference)
"""Trainium2 Bass kernel for grouped block-diagonal MLP (gnn_message_passing).

Computation: out[b, 3g+j] = sum_i x[b, 15g+i] * W[g, j, i]   (g<25, i<15, j<3)
Equivalent to out = x @ Wd where Wd is a [375, 75] block-diagonal matrix built
from the 25 stacked [3, 15] Linear weights (scattered per k_idx/v_idx).

Strategy (pure data parallel, 8 cores; memory-bound so minimize HBM traffic):
  - shard batch dim of x (262144 rows -> 8 x 32768), replicate Wd
  - host pre-transposes + casts each shard to fp16 as xt[375, 32768] so the
    contraction dim is already on partitions: no PE transposes, no PSUM
    round-trips for the inputs, and HBM read traffic is halved vs fp32.
    Tolerance is 2e-2; fp16 on N(0,1) data gives ~3e-4 relative error.
  - within each 4096-row block the columns are ordered (t, p) -> row p*32+t,
    which keeps every SBUF access contiguous AND gives the output DMA
    4800-byte contiguous runs per partition (full DMA bandwidth; runs under
    512B pay a 2x penalty).
  - per block: 3 input DMAs (one per 128-row chunk of the contraction dim),
    then 32 x (3 accumulating fp16 matmuls into an fp32 PSUM tile [128, 75]
    + one DVE copy that casts to fp16), then one output DMA.
  - host concatenates the 8 fp16 shards and upcasts to fp32.
"""

import numpy as np

B = 262144
NCORES = 8
B_CORE = B // NCORES  # 32768
F = 375   # input cols  (25 groups * 15)
O = 75    # output cols (25 groups * 3)
OUT_DIM = 75
CHUNKS = [(0, 128), (128, 128), (256, 119)]  # (offset, size) along F
T_BLK = 4096                 # rows per block
ST = T_BLK // 128            # 32 psum tiles per block
N_BLK = B_CORE // T_BLK      # 8

_compiled = {}


def _build_bass():
    import concourse.bass as bass
    import concourse.mybir as mybir
    import concourse.tile as tile
    from concourse import bacc

    f32 = mybir.dt.float32
    f16 = mybir.dt.float16
    nc = bacc.Bacc()
    xt_d = nc.dram_tensor("xt", [F, B_CORE], f16, kind="ExternalInput")
    w_d = nc.dram_tensor("wd", [3, 128, O], f16, kind="ExternalInput")
    o_d = nc.dram_tensor("out", [B_CORE, O], f16, kind="ExternalOutput")

    with tile.TileContext(nc) as tc:
        with (
            tc.tile_pool(name="const", bufs=1) as cpool,
            tc.tile_pool(name="xin", bufs=3) as xpool,
            tc.tile_pool(name="res", bufs=3) as rpool,
            tc.tile_pool(name="acc", bufs=7, space="PSUM") as pacc,
            tc.tile_pool(name="warm", bufs=1, space="PSUM") as pwarm,
        ):
            wd = cpool.tile([128, 3, O], f16)
            nc.sync.dma_start(wd[:], w_d[:].rearrange("c k n -> k c n"))

            # Absorb the wd DMA dependency so real matmuls only wait on
            # their own x-chunk DMA (PE instrs carry one semaphore wait).
            warm = pwarm.tile([O, O], f32)
            nc.tensor.matmul(
                warm[:], wd[:, 0, :], wd[:, 0, :], start=True, stop=True
            )

            for b in range(N_BLK):
                col0 = b * T_BLK
                xts = []
                for c, (off, sz) in enumerate(CHUNKS):
                    xt_sb = xpool.tile([128, T_BLK], f16, tag=f"xt{c}")
                    nc.sync.dma_start(
                        xt_sb[:sz, :], xt_d[off : off + sz, col0 : col0 + T_BLK]
                    )
                    xts.append(xt_sb)
                outb = rpool.tile([128, ST, O], f16)
                for t in range(ST):
                    ps = pacc.tile([128, O], f32)
                    for c, (off, sz) in enumerate(CHUNKS):
                        nc.tensor.matmul(
                            ps[:],
                            xts[c][:sz, t * 128 : (t + 1) * 128],
                            wd[:sz, c, :],
                            start=(c == 0),
                            stop=(c == 2),
                        )
                    nc.vector.tensor_copy(outb[:, t, :], ps[:])
                # column j = t*128 + p holds row p*ST + t of this block
                nc.sync.dma_start(
                    o_d[col0 : col0 + T_BLK, :].rearrange(
                        "(p t) f -> p t f", p=128
                    ),
                    outb[:],
                )
    nc.compile()
    return nc


def _get_nc():
    if "nc" not in _compiled:
        _compiled["nc"] = _build_bass()
    return _compiled["nc"]


def _build_wd_chunks(W, k_idx, v_idx):
    """Dense [3, 128, 75] chunked weight from stacked W (fp16)."""
    Wd = np.zeros((384, O), dtype=np.float32)
    kk = np.asarray(k_idx)
    vv = np.asarray(v_idx)
    Ww = np.asarray(W)
    # Wd[k_idx[g,i], v_idx[g,j]] = W[g, j, i]
    Wd[kk[:, :, None], vv[:, None, :]] = Ww.transpose(0, 2, 1)
    return np.ascontiguousarray(Wd.reshape(3, 128, O).astype(np.float16))


def _shard_xt(x16):
    """Per-core fp16 [375, 32768] with (t, p)-ordered columns per block."""
    # x16: [B, F] fp16. Within each 4096-row block, column j = t*128 + p
    # must hold row p*ST + t, i.e. layout [F, blocks, t, p].
    xs = x16.reshape(NCORES, N_BLK, 128, ST, F)
    xs = xs.transpose(0, 4, 1, 3, 2)  # [cores, F, blocks, t, p]
    return np.ascontiguousarray(xs.reshape(NCORES, F, B_CORE))


def kernel(x, W, k_idx, v_idx, **_unused):
    from concourse.bass_utils import run_bass_kernel_spmd

    x16 = np.asarray(x).astype(np.float16)
    xt = _shard_xt(x16)
    wd3 = _build_wd_chunks(W, k_idx, v_idx)
    nc = _get_nc()

    in_maps = [{"xt": xt[i], "wd": wd3} for i in range(NCORES)]
    res = run_bass_kernel_spmd(nc, in_maps, list(range(NCORES)))

    # Undo the per-block (t, p) column permutation while gathering.
    parts = []
    for i in range(NCORES):
        oc = res.results[i]["out"]  # [B_CORE, O], rows (block, p, t)
        oc = oc.reshape(N_BLK, 128, ST, O)
        parts.append(oc)
    got = np.stack(parts, axis=0)  # [cores, blocks, p, t, O]
    got = got.astype(np.float32).reshape(B, O)

    vflat = np.asarray(v_idx).reshape(-1)
    if vflat.shape[0] == OUT_DIM and np.array_equal(vflat, np.arange(OUT_DIM)):
        return np.ascontiguousarray(got)
    out = np.zeros((x.shape[0], OUT_DIM), dtype=np.float32)
    out[:, vflat] = got
    return out
